# revision 1
# baseline (speedup 1.0000x reference)
"""Trainium2 Bass kernel for CosineAttention:

    out = sigmoid((xn @ xn.T) @ x)   where xn = x / ||x_row||

Key algebraic optimization: reassociate (xn @ xn.T) @ x = xn @ (xn.T @ x).
G = xn.T @ x is [D, D] — the O(N^2 D) similarity matrix is never formed.
Total work drops from ~275 GFLOP to ~34 GFLOP.

Sharding: rows of x across 8 cores. Each core:
  1. loads its [N/8, D] row block, computes row norms + normalized rows
  2. computes partial G'_c = xn_c.T @ x_c - (c/8)*I  (f32 PSUM accum)
  3. one fp16 AllReduce(G') across the 8 cores
  4. out_c = sigmoid(xn_c @ G' + c*xn_c)
The host concatenates the 8 row blocks.

Precision: matmuls run in fp16 (full PE rate like bf16, but 10 mantissa
bits vs 7; all values here are comfortably in fp16 range). G's diagonal
(~256) dwarfs its off-diagonal entries (~3), so rounding G to fp16
would concentrate error on the diagonal; instead mm1 accumulates
-c/8 * shifted-identity into the PSUM so the AllReduce carries
G' = G - c*I (small entries), and the c*xn term is added back exactly
in f32 before the sigmoid. Measured ~2.6e-4 rel-L2 vs the f32
reference (plain bf16 is ~2.3e-3, f32r ~1.1e-4 at half the speed).

Schedule:
  - tiny warmup AllGather at t~0 absorbs the first-collective
    barrier/launch-skew + ncfw stream setup under the compute phase
  - mm1 is row-tile-outer (8 PSUM banks) so PE starts once ~3 tiles
    are loaded; norms use ACT Square-with-accumulate + 2 batched sqrts
    to avoid activation-table thrashing
  - xn.T is built with PE transposes (hidden in the AllReduce window)
  - mm2 is k-outer across 8 PSUM banks so it starts as soon as the
    first G' row-tile lands from the AllReduce
"""

import numpy as np

import concourse.bass as bass  # noqa: F401
import concourse.mybir as mybir
import concourse.tile as tile
from concourse import bacc
from concourse.bass_utils import run_bass_kernel_spmd
from concourse.masks import make_identity

F32 = mybir.dt.float32
F32R = mybir.dt.float32r
BF16 = mybir.dt.bfloat16
F16 = mybir.dt.float16
AFT = mybir.ActivationFunctionType

N, D = 8192, 1024
NCORES = 8
R = N // NCORES  # rows per core
P = 128
RT = R // P      # row tiles per core
KT = D // P      # contraction tiles (mm2) / G row tiles
FD = 512         # matmul moving free dim (one PSUM bank of f32)
NH = D // FD     # column halves
GROUPS = [list(range(NCORES))]
DIAG_C = 256.0   # ~mean of diag(G); exact in bf16


def _emit_body(tc, xb, out, mm_dt, ar_dt, ctx, use_diag=True):
    nc = tc.nc
    xb_t = xb.rearrange("(rt p) d -> rt p d", p=P)
    out_t = out.rearrange("(rt p) d -> rt p d", p=P)
    f32r_mode = mm_dt == F32R
    diag_trick = mm_dt in (BF16, F16) and use_diag

    persist = ctx.enter_context(tc.tile_pool(name="persist", bufs=1))
    load = ctx.enter_context(tc.tile_pool(name="load", bufs=3))
    small = ctx.enter_context(tc.tile_pool(name="small", bufs=1))
    gloc = ctx.enter_context(tc.tile_pool(name="gloc", bufs=3))
    gstage = ctx.enter_context(tc.tile_pool(name="gstage", bufs=3))
    ostage = ctx.enter_context(tc.tile_pool(name="ostage", bufs=8))
    ps = ctx.enter_context(tc.tile_pool(name="ps", bufs=1, space="PSUM"))
    dram = ctx.enter_context(tc.tile_pool(name="dram", bufs=1, space="DRAM"))

    # ---- warmup collective: the first collective in a NEFF pays the
    # cross-core launch-skew barrier plus ncfw stream setup (~20-30us).
    # A tiny AllGather (lowest-floor op) issued at t~0 absorbs that cost
    # under the compute phase so the real AllReduce starts promptly.
    # Input is uninitialized garbage; output is unused.
    w_in = dram.tile([P, 4], F32, tag="w_in")
    w_out = dram.tile([P * NCORES, 4], F32, tag="w_out")
    nc.gpsimd.collective_compute(
        "AllGather", mybir.AluOpType.bypass, replica_groups=GROUPS,
        ins=[w_in.opt()], outs=[w_out.opt()],
    )

    if f32r_mode:
        ident = persist.tile([P, P], F32, tag="ident")
        make_identity(nc, ident)
    if not f32r_mode:
        identb = persist.tile([P, P], mm_dt, tag="identb")
        make_identity(nc, identb)
    if diag_trick:
        # dsh[s]: [P, FD] bf16, -c * identity placed at columns [s*128,(s+1)*128)
        dsh = []
        for s in range(FD // P):
            t_dsh = persist.tile([P, FD], mm_dt, tag=f"dsh{s}", name=f"dsh{s}")
            nc.vector.memset(t_dsh, 0.0)
            nc.scalar.mul(t_dsh[:, s * P:(s + 1) * P], identb, -DIAG_C / NCORES)
            dsh.append(t_dsh)

    # ---- phase 0: load row block, norms, casts ----
    # Norms are batched: all squares accumulate into columns of one
    # [P, RT] tile, then ONE sqrt + ONE reciprocal — this avoids ACT
    # activation-table thrashing (table swaps cost ~1.4us each). cxn
    # (f32, needs xf) is deferred to the AllReduce window.
    # ACT runs per-tile Squares (with free-axis accumulate) and two
    # batched sqrts — [0:3] then [3:8] — so mm1's first matmuls issue
    # after ~3 tiles while keeping activation-table swaps to a minimum.
    # Casts/normalize run on DVE.
    xbr, xnr, xfs = [], [], []
    ss_all = small.tile([P, RT], F32, tag="ss_all")
    nrm_all = small.tile([P, RT], F32, tag="nrm_all")
    rn_all = small.tile([P, RT], F32, tag="rn_all")
    SPLIT = 3
    for rt in range(RT):
        xf = persist.tile([P, D], F32, tag=f"xf{rt}")
        nc.sync.dma_start(out=xf, in_=xb_t[rt])
        sq = load.tile([P, D], BF16, tag="sq")
        nc.scalar.activation(out=sq, in_=xf, func=AFT.Square,
                             accum_out=ss_all[:, rt:rt + 1])
        t_xbr = persist.tile([P, D], mm_dt, tag=f"xbr{rt}")
        nc.vector.tensor_copy(out=t_xbr, in_=xf)
        xbr.append(t_xbr)
        xfs.append(xf)
        if rt in (SPLIT - 1, RT - 1):
            lo, hi = (0, SPLIT) if rt == SPLIT - 1 else (SPLIT, RT)
            nc.scalar.sqrt(nrm_all[:, lo:hi], ss_all[:, lo:hi])
            nc.vector.reciprocal(rn_all[:, lo:hi], nrm_all[:, lo:hi])
            for rr in range(lo, hi):
                t_xnr = persist.tile([P, D], mm_dt, tag=f"xnr{rr}",
                                     name=f"xnr{rr}")
                nc.vector.tensor_scalar_mul(t_xnr, xfs[rr],
                                            rn_all[:, rr:rr + 1])
                xnr.append(t_xnr)

    # ---- phase 1: G'_c = xn_c.T @ x_c (- c*I/NCORES) ----
    # The AllReduce is split into two column-half chunks (separate DRAM
    # tensors): chunk 0 reduces while mm1's second wave runs, and mm2's
    # first z-wave (cols 0:512, full kt range) overlaps chunk 1.
    g_in = [dram.tile([D, FD], ar_dt, tag=f"g_in{h}", name=f"g_in{h}")
            for h in range(NH)]
    g_out = [dram.tile([D, FD], ar_dt, tag=f"g_out{h}", name=f"g_out{h}")
             for h in range(NH)]
    g_in_t = [g.rearrange("(mt p) f -> mt p f", p=P) for g in g_in]
    g_out_t = [g.rearrange("(kt p) f -> kt p f", p=P) for g in g_out]
    for nh in range(NH):
        psg = [ps.tile([P, FD], F32, tag=f"acc{mt}", name=f"psg{nh}_{mt}")
               for mt in range(KT)]
        for rt in range(RT):
            for mt in range(KT):
                has_diag = diag_trick and (mt // (FD // P) == nh)
                nc.tensor.matmul(
                    psg[mt],
                    lhsT=xnr[rt][:, mt * P:(mt + 1) * P],
                    rhs=xbr[rt][:, nh * FD:(nh + 1) * FD],
                    start=(rt == 0),
                    stop=(rt == RT - 1) and not has_diag,
                )
        if diag_trick:
            # diag-containing tiles get one extra matmul: += -c/8 * shifted I
            for mt in range(KT):
                if mt // (FD // P) == nh:
                    nc.tensor.matmul(
                        psg[mt], lhsT=identb, rhs=dsh[mt % (FD // P)],
                        start=False, stop=True,
                    )
        for mt in range(KT):
            gl = gloc.tile([P, FD], ar_dt, tag="gloc")
            nc.vector.tensor_copy(out=gl, in_=psg[mt])
            nc.sync.dma_start(out=g_in_t[nh][mt], in_=gl)
        nc.gpsimd.collective_compute(
            "AllReduce", mybir.AluOpType.add, replica_groups=GROUPS,
            ins=[g_in[nh].opt()], outs=[g_out[nh].opt()],
        )

    # ---- cxn = c*xn in f32, computed during the AllReduce window ----
    cxn = []
    if diag_trick:
        rc_all = small.tile([P, RT], F32, tag="rc_all")
        nc.scalar.mul(rc_all, rn_all, DIAG_C)
        for rt in range(RT):
            t_cxn = persist.tile([P, D], F32, tag=f"cxn{rt}")
            nc.vector.tensor_scalar_mul(t_cxn, xfs[rt], rc_all[:, rt:rt + 1])
            cxn.append(t_cxn)

    # ---- phase 1.5: xnT (DMA transpose for bf16, PE transpose for f32r) ----
    xnT = []
    for kt in range(KT):
        t_xnT = persist.tile([P, D], mm_dt, tag=f"xnT{kt}")
        for rt in range(RT):
            src = xnr[rt][:, kt * P:(kt + 1) * P]
            if f32r_mode:
                tpt = ps.tile([P, P], F32, tag=f"acc{rt % 2}", name=f"tp{kt}_{rt}")
                nc.tensor.transpose(tpt, src.bitcast(F32), ident)
            else:
                tpt = ps.tile([P, P], mm_dt, tag=f"acc{rt % 2}", name=f"tp{kt}_{rt}")
                nc.tensor.transpose(tpt, src, identb)
            nc.vector.tensor_copy(out=t_xnT[:, rt * P:(rt + 1) * P], in_=tpt)
        xnT.append(t_xnT)


    # ---- phases 3+4: per column half: load G chunk, mm2, sigmoid ----
    # k-outer: all 8 output banks accumulate in parallel, so each z-wave
    # starts as soon as its chunk's first G tile lands.
    for nh in range(NH):
        gr = []
        for kt in range(KT):
            t_gr = persist.tile([P, FD], mm_dt, tag=f"gr{nh}_{kt}",
                                name=f"gr{nh}_{kt}")
            if f32r_mode:
                gs = gstage.tile([P, FD], F32, tag="gs")
                nc.sync.dma_start(out=gs, in_=g_out_t[nh][kt])
                nc.vector.tensor_copy(out=t_gr, in_=gs)
            else:
                nc.sync.dma_start(out=t_gr, in_=g_out_t[nh][kt])
            gr.append(t_gr)
        psz = [ps.tile([P, FD], F32, tag=f"acc{mt}", name=f"psz{nh}_{mt}")
               for mt in range(RT)]
        for kt in range(KT):
            for mt in range(RT):
                nc.tensor.matmul(
                    psz[mt],
                    lhsT=xnT[kt][:, mt * P:(mt + 1) * P],
                    rhs=gr[kt],
                    start=(kt == 0),
                    stop=(kt == KT - 1),
                )
        for mt in range(RT):
            if diag_trick:
                nc.vector.tensor_add(
                    psz[mt], psz[mt], cxn[mt][:, nh * FD:(nh + 1) * FD]
                )
            ob = ostage.tile([P, FD], F32, tag="ob")
            nc.scalar.activation(out=ob, in_=psz[mt], func=AFT.Sigmoid)
            nc.sync.dma_start(out=out_t[mt][:, nh * FD:(nh + 1) * FD], in_=ob)


def build(mm_dt=F16, ar_dt=F16, use_diag=True):
    from contextlib import ExitStack

    nc = bacc.Bacc("TRN2", target_bir_lowering=False, debug=False,
                   num_devices=NCORES)
    xb = nc.dram_tensor("xb", [R, D], F32, kind="ExternalInput").ap()
    out = nc.dram_tensor("out", [R, D], F32, kind="ExternalOutput").ap()
    with tile.TileContext(nc) as tc:
        with ExitStack() as ctx:
            _emit_body(tc, xb, out, mm_dt, ar_dt, ctx, use_diag)
    nc.compile()
    return nc


_NC_CACHE = {}


def _get_nc(mm_dt=F16, ar_dt=F16):
    key = (str(mm_dt), str(ar_dt))
    if key not in _NC_CACHE:
        _NC_CACHE[key] = build(mm_dt, ar_dt)
    return _NC_CACHE[key]


def kernel(x: np.ndarray) -> np.ndarray:
    x = np.asarray(x, dtype=np.float32)
    assert x.shape == (N, D), x.shape
    nc = _get_nc()
    in_maps = [{"xb": x[c * R:(c + 1) * R]} for c in range(NCORES)]
    res = run_bass_kernel_spmd(nc, in_maps, list(range(NCORES)))
    return np.concatenate([res.results[c]["out"] for c in range(NCORES)], axis=0)



# revision 4
# speedup vs baseline: 1.0402x; 1.0402x over previous
"""Trainium2 Bass kernel for CosineAttention:

    out = sigmoid((xn @ xn.T) @ x)   where xn = x / ||x_row||

Key algebraic optimization: reassociate (xn @ xn.T) @ x = xn @ (xn.T @ x).
G = xn.T @ x is [D, D] — the O(N^2 D) similarity matrix is never formed.
Total work drops from ~275 GFLOP to ~34 GFLOP.

Sharding: rows of x across 8 cores. Each core:
  1. loads its [N/8, D] row block, computes row norms + normalized rows
  2. computes partial G'_c = xn_c.T @ x_c - (c/8)*I  (f32 PSUM accum)
  3. one fp16 AllReduce(G') across the 8 cores
  4. out_c = sigmoid(xn_c @ G' + c*xn_c)
The host concatenates the 8 row blocks.

Precision: matmuls run in fp16 (full PE rate like bf16, but 10 mantissa
bits vs 7; all values here are comfortably in fp16 range). G's diagonal
(~256) dwarfs its off-diagonal entries (~3), so rounding G to fp16
would concentrate error on the diagonal; instead mm1 accumulates
-c/8 * shifted-identity into the PSUM so the AllReduce carries
G' = G - c*I (small entries), and the c*xn term is added back exactly
in f32 before the sigmoid. Measured ~2.6e-4 rel-L2 vs the f32
reference (plain bf16 is ~2.3e-3, f32r ~1.1e-4 at half the speed).

Schedule:
  - tiny warmup AllGather at t~0 absorbs the first-collective
    barrier/launch-skew + ncfw stream setup under the compute phase
  - mm1 is row-tile-outer (8 PSUM banks) so PE starts once ~3 tiles
    are loaded; norms use ACT Square-with-accumulate + 2 batched sqrts
    to avoid activation-table thrashing
  - xn.T is built with PE transposes (hidden in the AllReduce window)
  - mm2 is k-outer across 8 PSUM banks so it starts as soon as the
    first G' row-tile lands from the AllReduce
"""

import numpy as np

import concourse.bass as bass  # noqa: F401
import concourse.mybir as mybir
import concourse.tile as tile
from concourse import bacc
from concourse.bass_utils import run_bass_kernel_spmd
from concourse.masks import make_identity

F32 = mybir.dt.float32
F32R = mybir.dt.float32r
BF16 = mybir.dt.bfloat16
F16 = mybir.dt.float16
AFT = mybir.ActivationFunctionType

N, D = 8192, 1024
NCORES = 8
R = N // NCORES  # rows per core
P = 128
RT = R // P      # row tiles per core
KT = D // P      # contraction tiles (mm2) / G row tiles
FD = 512         # matmul moving free dim (one PSUM bank of f32)
NH = D // FD     # column halves
GROUPS = [list(range(NCORES))]
DIAG_C = 256.0   # ~mean of diag(G); exact in bf16


def _emit_body(tc, xb, out, mm_dt, ar_dt, ctx, use_diag=True):
    nc = tc.nc
    xb_t = xb.rearrange("(rt p) d -> rt p d", p=P)
    out_t = out.rearrange("(rt p) d -> rt p d", p=P)
    f32r_mode = mm_dt == F32R
    diag_trick = mm_dt in (BF16, F16) and use_diag

    persist = ctx.enter_context(tc.tile_pool(name="persist", bufs=1))
    load = ctx.enter_context(tc.tile_pool(name="load", bufs=3))
    small = ctx.enter_context(tc.tile_pool(name="small", bufs=1))
    gloc = ctx.enter_context(tc.tile_pool(name="gloc", bufs=3))
    gstage = ctx.enter_context(tc.tile_pool(name="gstage", bufs=3))
    ostage = ctx.enter_context(tc.tile_pool(name="ostage", bufs=8))
    ps = ctx.enter_context(tc.tile_pool(name="ps", bufs=1, space="PSUM"))
    dram = ctx.enter_context(tc.tile_pool(name="dram", bufs=1, space="DRAM"))

    # (warmup collective removed — probing whether the first-collective
    # setup can ride under compute without it)

    if f32r_mode:
        ident = persist.tile([P, P], F32, tag="ident")
        make_identity(nc, ident)
    if not f32r_mode:
        identb = persist.tile([P, P], mm_dt, tag="identb")
        make_identity(nc, identb)
    if diag_trick:
        # dsh[s]: [P, FD] bf16, -c * identity placed at columns [s*128,(s+1)*128)
        dsh = []
        for s in range(FD // P):
            t_dsh = persist.tile([P, FD], mm_dt, tag=f"dsh{s}", name=f"dsh{s}")
            nc.vector.memset(t_dsh, 0.0)
            nc.scalar.mul(t_dsh[:, s * P:(s + 1) * P], identb, -DIAG_C / NCORES)
            dsh.append(t_dsh)

    # ---- phase 0: load row block, norms, casts ----
    # Norms are batched: all squares accumulate into columns of one
    # [P, RT] tile, then ONE sqrt + ONE reciprocal — this avoids ACT
    # activation-table thrashing (table swaps cost ~1.4us each). cxn
    # (f32, needs xf) is deferred to the AllReduce window.
    # ACT runs per-tile Squares (with free-axis accumulate) and two
    # batched sqrts — [0:3] then [3:8] — so mm1's first matmuls issue
    # after ~3 tiles while keeping activation-table swaps to a minimum.
    # Casts/normalize run on DVE.
    xbr, xnr, xfs = [], [], []
    ss_all = small.tile([P, RT], F32, tag="ss_all")
    nrm_all = small.tile([P, RT], F32, tag="nrm_all")
    rn_all = small.tile([P, RT], F32, tag="rn_all")
    SPLIT = 3
    for rt in range(RT):
        xf = persist.tile([P, D], F32, tag=f"xf{rt}")
        nc.sync.dma_start(out=xf, in_=xb_t[rt])
        sq = load.tile([P, D], BF16, tag="sq")
        nc.scalar.activation(out=sq, in_=xf, func=AFT.Square,
                             accum_out=ss_all[:, rt:rt + 1])
        t_xbr = persist.tile([P, D], mm_dt, tag=f"xbr{rt}")
        nc.vector.tensor_copy(out=t_xbr, in_=xf)
        xbr.append(t_xbr)
        xfs.append(xf)
        if rt in (SPLIT - 1, RT - 1):
            lo, hi = (0, SPLIT) if rt == SPLIT - 1 else (SPLIT, RT)
            nc.scalar.sqrt(nrm_all[:, lo:hi], ss_all[:, lo:hi])
            nc.vector.reciprocal(rn_all[:, lo:hi], nrm_all[:, lo:hi])
            for rr in range(lo, hi):
                t_xnr = persist.tile([P, D], mm_dt, tag=f"xnr{rr}",
                                     name=f"xnr{rr}")
                nc.vector.tensor_scalar_mul(t_xnr, xfs[rr],
                                            rn_all[:, rr:rr + 1])
                xnr.append(t_xnr)

    # ---- phase 1: G'_c = xn_c.T @ x_c (- c*I/NCORES) ----
    # The AllReduce is split into two column-half chunks (separate DRAM
    # tensors): chunk 0 reduces while mm1's second wave runs, and mm2's
    # first z-wave (cols 0:512, full kt range) overlaps chunk 1.
    g_in = [dram.tile([D, FD], ar_dt, tag=f"g_in{h}", name=f"g_in{h}")
            for h in range(NH)]
    g_out = [dram.tile([D, FD], ar_dt, tag=f"g_out{h}", name=f"g_out{h}",
                       addr_space="Shared")
             for h in range(NH)]
    g_in_t = [g.rearrange("(mt p) f -> mt p f", p=P) for g in g_in]
    g_out_t = [g.rearrange("(kt p) f -> kt p f", p=P) for g in g_out]
    for nh in range(NH):
        psg = [ps.tile([P, FD], F32, tag=f"acc{mt}", name=f"psg{nh}_{mt}")
               for mt in range(KT)]
        for rt in range(RT):
            for mt in range(KT):
                has_diag = diag_trick and (mt // (FD // P) == nh)
                nc.tensor.matmul(
                    psg[mt],
                    lhsT=xnr[rt][:, mt * P:(mt + 1) * P],
                    rhs=xbr[rt][:, nh * FD:(nh + 1) * FD],
                    start=(rt == 0),
                    stop=(rt == RT - 1) and not has_diag,
                )
        if diag_trick:
            # diag-containing tiles get one extra matmul: += -c/8 * shifted I
            for mt in range(KT):
                if mt // (FD // P) == nh:
                    nc.tensor.matmul(
                        psg[mt], lhsT=identb, rhs=dsh[mt % (FD // P)],
                        start=False, stop=True,
                    )
        for mt in range(KT):
            gl = gloc.tile([P, FD], ar_dt, tag="gloc")
            nc.vector.tensor_copy(out=gl, in_=psg[mt])
            nc.sync.dma_start(out=g_in_t[nh][mt], in_=gl)
        nc.gpsimd.collective_compute(
            "AllReduce", mybir.AluOpType.add, replica_groups=GROUPS,
            ins=[g_in[nh].opt()], outs=[g_out[nh].opt()],
        )

    # ---- cxn = c*xn in f32, computed during the AllReduce window ----
    cxn = []
    if diag_trick:
        rc_all = small.tile([P, RT], F32, tag="rc_all")
        nc.scalar.mul(rc_all, rn_all, DIAG_C)
        for rt in range(RT):
            t_cxn = persist.tile([P, D], F32, tag=f"cxn{rt}")
            nc.vector.tensor_scalar_mul(t_cxn, xfs[rt], rc_all[:, rt:rt + 1])
            cxn.append(t_cxn)

    # ---- phase 1.5: xnT (DMA transpose for bf16, PE transpose for f32r) ----
    xnT = []
    for kt in range(KT):
        t_xnT = persist.tile([P, D], mm_dt, tag=f"xnT{kt}")
        for rt in range(RT):
            src = xnr[rt][:, kt * P:(kt + 1) * P]
            if f32r_mode:
                tpt = ps.tile([P, P], F32, tag=f"acc{rt % 2}", name=f"tp{kt}_{rt}")
                nc.tensor.transpose(tpt, src.bitcast(F32), ident)
            else:
                tpt = ps.tile([P, P], mm_dt, tag=f"acc{rt % 2}", name=f"tp{kt}_{rt}")
                nc.tensor.transpose(tpt, src, identb)
            nc.vector.tensor_copy(out=t_xnT[:, rt * P:(rt + 1) * P], in_=tpt)
        xnT.append(t_xnT)


    # ---- phases 3+4: per column half: load G chunk, mm2, sigmoid ----
    # k-outer: all 8 output banks accumulate in parallel, so each z-wave
    # starts as soon as its chunk's first G tile lands.
    for nh in range(NH):
        gr = []
        for kt in range(KT):
            t_gr = persist.tile([P, FD], mm_dt, tag=f"gr{nh}_{kt}",
                                name=f"gr{nh}_{kt}")
            if f32r_mode:
                gs = gstage.tile([P, FD], F32, tag="gs")
                nc.sync.dma_start(out=gs, in_=g_out_t[nh][kt])
                nc.vector.tensor_copy(out=t_gr, in_=gs)
            else:
                nc.sync.dma_start(out=t_gr, in_=g_out_t[nh][kt])
            gr.append(t_gr)
        psz = [ps.tile([P, FD], F32, tag=f"acc{mt}", name=f"psz{nh}_{mt}")
               for mt in range(RT)]
        for kt in range(KT):
            for mt in range(RT):
                nc.tensor.matmul(
                    psz[mt],
                    lhsT=xnT[kt][:, mt * P:(mt + 1) * P],
                    rhs=gr[kt],
                    start=(kt == 0),
                    stop=(kt == KT - 1),
                )
        for mt in range(RT):
            if diag_trick:
                nc.vector.tensor_add(
                    psz[mt], psz[mt], cxn[mt][:, nh * FD:(nh + 1) * FD]
                )
            ob = ostage.tile([P, FD], F32, tag="ob")
            nc.scalar.activation(out=ob, in_=psz[mt], func=AFT.Sigmoid)
            # alternate the two HWDGE issue FIFOs (Sync / Scalar) so the
            # 8 bunched end-of-wave stores don't serialize on one ring
            eng = nc.sync if mt % 2 == 0 else nc.scalar
            eng.dma_start(out=out_t[mt][:, nh * FD:(nh + 1) * FD], in_=ob)


def build(mm_dt=F16, ar_dt=F16, use_diag=True):
    from contextlib import ExitStack

    nc = bacc.Bacc("TRN2", target_bir_lowering=False, debug=False,
                   num_devices=NCORES)
    xb = nc.dram_tensor("xb", [R, D], F32, kind="ExternalInput").ap()
    out = nc.dram_tensor("out", [R, D], F32, kind="ExternalOutput").ap()
    with tile.TileContext(nc) as tc:
        with ExitStack() as ctx:
            _emit_body(tc, xb, out, mm_dt, ar_dt, ctx, use_diag)
    nc.compile()
    return nc


_NC_CACHE = {}


def _get_nc(mm_dt=F16, ar_dt=F16):
    key = (str(mm_dt), str(ar_dt))
    if key not in _NC_CACHE:
        _NC_CACHE[key] = build(mm_dt, ar_dt)
    return _NC_CACHE[key]


def kernel(x: np.ndarray) -> np.ndarray:
    x = np.asarray(x, dtype=np.float32)
    assert x.shape == (N, D), x.shape
    nc = _get_nc()
    in_maps = [{"xb": x[c * R:(c + 1) * R]} for c in range(NCORES)]
    res = run_bass_kernel_spmd(nc, in_maps, list(range(NCORES)))
    return np.concatenate([res.results[c]["out"] for c in range(NCORES)], axis=0)



# revision 6
# speedup vs baseline: 1.1576x; 1.1129x over previous
"""Trainium2 Bass kernel for CosineAttention:

    out = sigmoid((xn @ xn.T) @ x)   where xn = x / ||x_row||

Key algebraic optimization: reassociate (xn @ xn.T) @ x = xn @ (xn.T @ x).
G = xn.T @ x is [D, D] - the O(N^2 D) similarity matrix is never formed.

Sharding: rows of x across 8 cores. Each core:
  1. loads its [N/8, D] row block, computes row norms + normalized rows
  2. computes partial G'_c = xn_c.T @ x_c - (c/8)*I  (f32 PSUM accum)
  3. AllReduce the result across the 8 cores (fp16 payload)
  4. out_c = sigmoid(xn_c @ G' + c*xn_c)
The host concatenates the 8 row blocks.

G is symmetric, so only the left column-half (cols 0:512, 1MB) and the
lower-right quadrant (rows/cols 512:1024, 512KB) are AllReduced; the
upper-right quadrant is reconstructed on-chip by PE-transposing the
lower-left blocks of the first AllReduce result. This shrinks the
second (serialized) collective to half size and skips 32 of mm1's
matmuls.

Precision: matmuls in fp16; x is cast to fp16 once and norms/cxn are
derived from the fp16 copy (costs ~1e-3 rel err total, gate is 2e-2).
G's diagonal (~256) would dominate fp16 rounding of the AllReduce
payload, so mm1 accumulates -c/8*I into the PSUM (diag trick) and the
c*xn term is added back in f32 before the sigmoid.

Schedule: loads stream in half-tiles; norms use 2 batched sqrts; mm1
paces the loads; both AllReduce doorbells fire as early as the data
dependency allows (~31us / ~38us); xnT transposes + cxn muls hide in
the AR1 window; mm2 col-half 0 + the quadrant reconstruction hide in
the AR2 window; stores alternate the two HWDGE rings.
"""

import numpy as np

import concourse.bass as bass  # noqa: F401
import concourse.mybir as mybir
import concourse.tile as tile
from concourse import bacc
from concourse.bass_utils import run_bass_kernel_spmd
from concourse.masks import make_identity

F32 = mybir.dt.float32
BF16 = mybir.dt.bfloat16
F16 = mybir.dt.float16
AFT = mybir.ActivationFunctionType

N, D = 8192, 1024
NCORES = 8
R = N // NCORES  # rows per core
P = 128
RT = R // P      # row tiles per core
KT = D // P      # contraction tiles (mm2) / G row tiles
FD = 512         # matmul moving free dim (one PSUM bank of f32)
NH = D // FD     # column halves
QT = KT // 2     # tiles per half (4)
GROUPS = [list(range(NCORES))]
DIAG_C = 256.0   # ~mean of diag(G); exact in fp16


def _emit_body(tc, xb, out, ctx):
    nc = tc.nc
    mm_dt = F16
    xb_t = xb.rearrange("(rt p) d -> rt p d", p=P)
    out_t = out.rearrange("(rt p) d -> rt p d", p=P)

    persist = ctx.enter_context(tc.tile_pool(name="persist", bufs=1))
    load = ctx.enter_context(tc.tile_pool(name="load", bufs=3))
    small = ctx.enter_context(tc.tile_pool(name="small", bufs=1))
    ostage = ctx.enter_context(tc.tile_pool(name="ostage", bufs=8))
    ps = ctx.enter_context(tc.tile_pool(name="ps", bufs=1, space="PSUM"))
    dram = ctx.enter_context(tc.tile_pool(name="dram", bufs=1, space="DRAM"))

    # ---- warmup collective: absorbs first-collective ncfw setup +
    # cross-core launch skew before the real AllReduces need the TOPSP.
    w_in = dram.tile([P, 4], F32, tag="w_in")
    w_out = dram.tile([P * NCORES, 4], F32, tag="w_out", addr_space="Shared")
    nc.gpsimd.collective_compute(
        "AllGather", mybir.AluOpType.bypass, replica_groups=GROUPS,
        ins=[w_in.opt()], outs=[w_out.opt()],
    )

    identb = persist.tile([P, P], mm_dt, tag="identb")
    make_identity(nc, identb)
    # dsh[s]: [P, FD] f16, -c/8 * identity placed at columns [s*128,(s+1)*128)
    dsh = []
    for s in range(FD // P):
        t_dsh = persist.tile([P, FD], mm_dt, tag=f"dsh{s}", name=f"dsh{s}")
        nc.vector.memset(t_dsh, 0.0)
        nc.scalar.mul(t_dsh[:, s * P:(s + 1) * P], identb, -DIAG_C / NCORES)
        dsh.append(t_dsh)

    # ---- phase 0: stream loads, cast to fp16, norms ----
    # Loads arrive as [P, FD] halves. Per row tile: DVE casts f32->f16
    # (xbr), ACT Squares straight off the f32 halves with free-axis
    # accumulate. Two sqrt batches ({0,1} then {2..7}) keep ACT
    # activation-table swaps to 3. xn = xbr * (1/norm) on DVE at fp16
    # rate. cxn is deferred to the AllReduce window.
    xbr, xnr = [], []
    ssA = small.tile([P, RT], F32, tag="ssA")
    ssB = small.tile([P, RT], F32, tag="ssB")
    ss_all = small.tile([P, RT], F32, tag="ss_all")
    nrm_all = small.tile([P, RT], F32, tag="nrm_all")
    rn_all = small.tile([P, RT], F32, tag="rn_all")
    SPLIT = 2
    for rt in range(RT):
        xf = load.tile([P, D], F32, tag="xf")
        nc.sync.dma_start(out=xf[:, :FD], in_=xb_t[rt][:, :FD])
        nc.sync.dma_start(out=xf[:, FD:], in_=xb_t[rt][:, FD:])
        sqa = load.tile([P, FD], BF16, tag="sqa")
        nc.scalar.activation(out=sqa, in_=xf[:, :FD], func=AFT.Square,
                             accum_out=ssA[:, rt:rt + 1])
        sqb = load.tile([P, FD], BF16, tag="sqb")
        nc.scalar.activation(out=sqb, in_=xf[:, FD:], func=AFT.Square,
                             accum_out=ssB[:, rt:rt + 1])
        t_xbr = persist.tile([P, D], mm_dt, tag=f"xbr{rt}", name=f"xbr{rt}")
        nc.vector.tensor_copy(out=t_xbr, in_=xf)
        xbr.append(t_xbr)
        if rt in (SPLIT - 1, RT - 1):
            lo, hi = (0, SPLIT) if rt == SPLIT - 1 else (SPLIT, RT)
            nc.vector.tensor_add(ss_all[:, lo:hi], ssA[:, lo:hi],
                                 ssB[:, lo:hi])
            nc.scalar.sqrt(nrm_all[:, lo:hi], ss_all[:, lo:hi])
            nc.vector.reciprocal(rn_all[:, lo:hi], nrm_all[:, lo:hi])
            for rr in range(lo, hi):
                t_xnr = persist.tile([P, D], mm_dt, tag=f"xnr{rr}",
                                     name=f"xnr{rr}")
                nc.vector.tensor_scalar_mul(t_xnr, xbr[rr],
                                            rn_all[:, rr:rr + 1])
                xnr.append(t_xnr)

    # ---- phase 1a: G' cols 0:512 = xn_c.T @ x_c[:, 0:512] (- c/8*I) ----
    g_in0 = dram.tile([D, FD], mm_dt, tag="g_in0")
    g_out0 = dram.tile([D, FD], mm_dt, tag="g_out0", addr_space="Shared")
    g_in0_g = g_in0.rearrange("(g q p) f -> g p q f", p=P, q=QT)
    g_out0_t = g_out0.rearrange("(kt p) f -> kt p f", p=P)

    psg0 = [ps.tile([P, FD], F32, tag=f"acc{mt}", name=f"psg0_{mt}")
            for mt in range(KT)]
    for rt in range(RT):
        for mt in range(KT):
            nc.tensor.matmul(
                psg0[mt],
                lhsT=xnr[rt][:, mt * P:(mt + 1) * P],
                rhs=xbr[rt][:, 0:FD],
                start=(rt == 0),
                stop=(rt == RT - 1) and mt >= QT,
            )
    for mt in range(QT):
        # diag blocks live at mt 0..3 for the left column-half
        nc.tensor.matmul(psg0[mt], lhsT=identb, rhs=dsh[mt],
                         start=False, stop=True)

    gA = [persist.tile([P, QT, FD], mm_dt, tag=f"gA{g}", name=f"gA{g}")
          for g in range(2)]
    for mt in range(KT):
        g, q = divmod(mt, QT)
        eng = nc.vector if mt < QT else nc.scalar
        if eng is nc.vector:
            nc.vector.tensor_copy(out=gA[g][:, q, :], in_=psg0[mt])
        else:
            nc.scalar.copy(out=gA[g][:, q, :], in_=psg0[mt])
    for g in range(2):
        nc.sync.dma_start(out=g_in0_g[g], in_=gA[g])
    nc.gpsimd.collective_compute(
        "AllReduce", mybir.AluOpType.add, replica_groups=GROUPS,
        ins=[g_in0.opt()], outs=[g_out0.opt()],
    )

    # ---- phase 1b: G' lower-right quadrant rows/cols 512:1024 ----
    g_in1 = dram.tile([FD, FD], mm_dt, tag="g_in1")
    g_out1 = dram.tile([FD, FD], mm_dt, tag="g_out1", addr_space="Shared")
    g_in1_g = g_in1.rearrange("(q p) f -> p q f", p=P)
    g_out1_t = g_out1.rearrange("(q p) f -> q p f", p=P)

    psg1 = [ps.tile([P, FD], F32, tag=f"acc{QT + q}", name=f"psg1_{q}")
            for q in range(QT)]
    for rt in range(RT):
        for q in range(QT):
            nc.tensor.matmul(
                psg1[q],
                lhsT=xnr[rt][:, (QT + q) * P:(QT + q + 1) * P],
                rhs=xbr[rt][:, FD:],
                start=(rt == 0),
                stop=False,
            )
    for q in range(QT):
        nc.tensor.matmul(psg1[q], lhsT=identb, rhs=dsh[q],
                         start=False, stop=True)
    gB = persist.tile([P, QT, FD], mm_dt, tag="gB")
    for q in range(QT):
        eng_v = (q % 2 == 0)
        if eng_v:
            nc.vector.tensor_copy(out=gB[:, q, :], in_=psg1[q])
        else:
            nc.scalar.copy(out=gB[:, q, :], in_=psg1[q])
    nc.sync.dma_start(out=g_in1_g, in_=gB)
    nc.gpsimd.collective_compute(
        "AllReduce", mybir.AluOpType.add, replica_groups=GROUPS,
        ins=[g_in1.opt()], outs=[g_out1.opt()],
    )

    # ---- phase 1c (hidden in AR windows): cxn + xnT transposes ----
    rc_all = small.tile([P, RT], F32, tag="rc_all")
    nc.scalar.mul(rc_all, rn_all, DIAG_C)
    cxn = []
    for rt in range(RT):
        t_cxn = persist.tile([P, D], F32, tag=f"cxn{rt}", name=f"cxn{rt}")
        nc.vector.tensor_scalar_mul(t_cxn, xbr[rt], rc_all[:, rt:rt + 1])
        cxn.append(t_cxn)

    xnT = []
    for kt in range(KT):
        t_xnT = persist.tile([P, D], mm_dt, tag=f"xnT{kt}", name=f"xnT{kt}")
        for rt in range(RT):
            src = xnr[rt][:, kt * P:(kt + 1) * P]
            tpt = ps.tile([P, P], mm_dt, tag=f"acc{rt % 2}",
                          name=f"tp{kt}_{rt}")
            nc.tensor.transpose(tpt, src, identb)
            if rt % 2 == 0:
                nc.vector.tensor_copy(out=t_xnT[:, rt * P:(rt + 1) * P],
                                      in_=tpt)
            else:
                nc.scalar.copy(out=t_xnT[:, rt * P:(rt + 1) * P], in_=tpt)
        xnT.append(t_xnT)

    # ---- phase 2: load G chunk 0, reconstruct upper-right, mm2 ----
    gr0 = [persist.tile([P, FD], mm_dt, tag=f"gr0_{kt}", name=f"gr0_{kt}")
           for kt in range(KT)]
    for kt in range(KT):
        nc.sync.dma_start(out=gr0[kt], in_=g_out0_t[kt])

    # upper-right quadrant G'[0:512, 512:1024] = transpose of
    # G'[512:1024, 0:512] (= gr0[4..7]) block-wise
    grT = [persist.tile([P, FD], mm_dt, tag=f"grT{q}", name=f"grT{q}")
           for q in range(QT)]
    for q in range(QT):          # target row-block q (cols 512:1024)
        for b in range(QT):      # source row-block 4+b
            tpq = ps.tile([P, P], mm_dt, tag=f"acc{b % 2}",
                          name=f"tpq{q}_{b}")
            nc.tensor.transpose(tpq, gr0[QT + b][:, q * P:(q + 1) * P],
                                identb)
            if b % 2 == 0:
                nc.vector.tensor_copy(out=grT[q][:, b * P:(b + 1) * P],
                                      in_=tpq)
            else:
                nc.scalar.copy(out=grT[q][:, b * P:(b + 1) * P], in_=tpq)

    def mm2_half(nh, gr):
        psz = [ps.tile([P, FD], F32, tag=f"acc{mt}", name=f"psz{nh}_{mt}")
               for mt in range(RT)]
        for kt in range(KT):
            for mt in range(RT):
                nc.tensor.matmul(
                    psz[mt],
                    lhsT=xnT[kt][:, mt * P:(mt + 1) * P],
                    rhs=gr[kt],
                    start=(kt == 0),
                    stop=(kt == KT - 1),
                )
        for mt in range(RT):
            nc.vector.tensor_add(
                psz[mt], psz[mt], cxn[mt][:, nh * FD:(nh + 1) * FD]
            )
            ob = ostage.tile([P, FD], F32, tag="ob")
            nc.scalar.activation(out=ob, in_=psz[mt], func=AFT.Sigmoid)
            eng = nc.sync if mt % 2 == 0 else nc.scalar
            eng.dma_start(out=out_t[mt][:, nh * FD:(nh + 1) * FD], in_=ob)

    mm2_half(0, gr0)

    # ---- phase 3: after AR2, assemble col-half 1 rhs and finish ----
    grq = [persist.tile([P, FD], mm_dt, tag=f"grq{q}", name=f"grq{q}")
           for q in range(QT)]
    for q in range(QT):
        nc.sync.dma_start(out=grq[q], in_=g_out1_t[q])
    mm2_half(1, grT + grq)


def build():
    from contextlib import ExitStack

    nc = bacc.Bacc("TRN2", target_bir_lowering=False, debug=False,
                   num_devices=NCORES)
    xb = nc.dram_tensor("xb", [R, D], F32, kind="ExternalInput").ap()
    out = nc.dram_tensor("out", [R, D], F32, kind="ExternalOutput").ap()
    with tile.TileContext(nc) as tc:
        with ExitStack() as ctx:
            _emit_body(tc, xb, out, ctx)
    nc.compile()
    return nc


_NC_CACHE = {}


def _get_nc():
    if "nc" not in _NC_CACHE:
        _NC_CACHE["nc"] = build()
    return _NC_CACHE["nc"]


def kernel(x: np.ndarray) -> np.ndarray:
    x = np.asarray(x, dtype=np.float32)
    assert x.shape == (N, D), x.shape
    nc = _get_nc()
    in_maps = [{"xb": x[c * R:(c + 1) * R]} for c in range(NCORES)]
    res = run_bass_kernel_spmd(nc, in_maps, list(range(NCORES)))
    return np.concatenate([res.results[c]["out"] for c in range(NCORES)], axis=0)
